# revision 3
# baseline (speedup 1.0000x reference)
"""Correlation layer (FlowNet-style) on 8 Trainium2 NeuronCores.

Data-parallel over batch (one element per core). Per core, banded-Gram
matmuls with displacement packing:
  - lhsT = x1 rows [24g-8+rho'' : +32) x 4 w-cols -> M = 32*4 = 128
  - rhs  = x2p rows [24g : 24g+24) x 12 padded-w cols -> N = 24*12 = 288
  - psum[m, n] useful iff rho'' = rr - di + 4 and u - ww in [0, 9).

v2 (resident-slab): the previous version streamed host-padded slabs per
group (x1 9.43MB with 33% row duplication across 32-row windows at
24-row stride + 24 zero rows; x2p 7.37MB with pad rows/cols), 23.4MB
HBM/core/exec ~ 65us DMA floor at 358 GB/s/core, measured 63-69us.
Now both operands live in persistent SBUF slabs sized for the whole
feature map:
  - x1 slab [128c, 48wb, 152rho x 4ww] (57KB/partition bf16): group g's
    stationary operand for block wb is the contiguous 128-elem slice
    rows 24g..24g+32. HBM stores only the 128 real rows in the same
    blocked order (6.29MB); the 24 zero-pad rows are memset once per
    NEFF and never rewritten.
  - x2 slab [128c, 144r, 200u] (56.25KB/partition): rhs for (g, wb) is
    rows 24g..24g+24, cols 4wb..4wb+12. HBM stores the natural 128x192
    map (6.29MB); pad rows/cols memset once.
  - Steady state reloads in 6 row-chunks per tensor per exec, each
    issued after the group that last reads that region, so the WAR dep
    (chunk c write <- groups c-1, c matmul reads) costs no stall and
    ~1 exec-period of pipeline slack.
Band output unchanged (6.64MB bf16): PSUM evac copies alternate
DVE/ACT, band-window DMAs issue on SP/Pool ("yggg"). Total ~19.2MB
HBM/core/exec -> ~53.7us DMA floor.
"""

import math
import numpy as np
from contextlib import ExitStack

B, C, H, W = 8, 128, 128, 192
MD = 4
NDISP = 81

R = 24            # x2p rows per group
NG = 6            # row groups
WW = 4            # output w-cols per block
NWB = W // WW     # 48 blocks
UB = WW + 8       # 12 rhs cols per block
NMM = R * UB      # 288 matmul free size
HP = 152          # x1 padded rows: 8 zero + 128 + 16 zero
X2C = W + 8       # 200
X2R = NG * R      # 144 x2 slab rows (136 logical + 8 zero tail)
WIN = 36                   # band window partitions per rr
WINR = 2                   # rr values merged per band-window DMA
NHALF = 1                  # staging chunks per group (windows launch per chunk)
WBH = NWB // NHALF         # w-blocks per staging chunk
RSH = R * WBH * UB         # staging elems/partition per chunk
NQ = R // WINR             # window DMAs per chunk
WQP = WIN + 4 * (WINR - 1)     # partitions per merged window
WQE_H = WINR * WBH * UB        # free elems per merged window per chunk
GRP_OUT = NHALF * NQ * WQP * WQE_H
OUTSZ = NG * GRP_OUT

IN_DTYPE = "bfloat16"     # "bfloat16" | "float32"
MM_DTYPE = "bfloat16"     # "bfloat16" | "float32" | "float32r" (must match IN unless f32->f32r)
BAND_DTYPE = "bfloat16"   # "float32" | "bfloat16"
X1_MODE = "resident"
EVAC = "vs"               # per-copy engine cycle: v=DVE, s=ACT (Pool can't read PSUM)
WDMA = "yggg"             # window-DMA issue engine cycle: y=SP, s=ACT, g=Pool(SWDGE)
PSGRP = 2                 # matmuls per PSUM tile (PSGRP banks; ring of 8/PSGRP)

_CACHE = {}


def _build(in_dtype_s, mm_dtype_s, band_dtype_s, evac, wdma, x1_mode, nrep=1):
    import concourse.bass as bass
    import concourse.tile as tile
    from concourse import bacc, mybir

    dtmap = {"bfloat16": mybir.dt.bfloat16, "float32": mybir.dt.float32,
             "float32r": mybir.dt.float32r}
    in_dt = dtmap[in_dtype_s]
    band_dt = dtmap[band_dtype_s]
    f32 = mybir.dt.float32

    nc = bacc.Bacc("TRN2", target_bir_lowering=False, debug=False, num_devices=8)
    if nrep > 1:
        # the NEFF cache hashes HLO structure only (not the embedded BIR);
        # an nrep-dependent input shape keeps repeat variants distinct
        nc.dram_tensor("reptag", [1, 8 * nrep], mybir.dt.float32,
                       kind="ExternalInput")
    # x1 blocked (wb, h, ww): real rows only; zero pads live in SBUF
    x1d = nc.dram_tensor("x1s", [128, NWB, H * WW], in_dt, kind="ExternalInput")
    # x2 natural (h, w)
    x2d = nc.dram_tensor("x2n", [128, H, W], in_dt, kind="ExternalInput")
    outd = nc.dram_tensor("band", [OUTSZ], band_dt, kind="ExternalOutput")

    with tile.TileContext(nc) as tc, ExitStack() as ctx:
        slabpool = ctx.enter_context(tc.tile_pool(name="slab", bufs=1))
        pspool = ctx.enter_context(tc.tile_pool(name="ps", bufs=8 // PSGRP, space="PSUM"))
        stpool = ctx.enter_context(tc.tile_pool(name="st", bufs=3))

        x1t = slabpool.tile([128, NWB, HP * WW], in_dt, tag="x1slab")
        x2t = slabpool.tile([128, X2R, X2C], in_dt, tag="x2slab")

        # one-time zero pads (never rewritten; reload chunks cover real rows only)
        nc.gpsimd.memset(x1t[:, :, 0:8 * WW], 0.0)            # rho'' 0..8
        nc.gpsimd.memset(x1t[:, :, 136 * WW:HP * WW], 0.0)    # rho'' 136..152
        nc.vector.memset(x2t[:, 0:4, :], 0.0)                 # top pad rows
        nc.vector.memset(x2t[:, 132:X2R, :], 0.0)             # bottom pad + tail
        nc.gpsimd.memset(x2t[:, 4:132, 0:4], 0.0)             # left pad cols
        nc.vector.memset(x2t[:, 4:132, 196:X2C], 0.0)         # right pad cols

        def load_chunks(c):
            # x1 slab rows [24c, 24c+24) clipped to real rows (8..136)
            r0, r1 = max(8, 24 * c), min(136, 24 * c + 24)
            nc.sync.dma_start(x1t[:, :, r0 * WW:r1 * WW],
                              x1d.ap()[:, :, (r0 - 8) * WW:(r1 - 8) * WW])
            # x2 slab rows [24c, 24c+24) clipped to real rows (4..132)
            q0, q1 = max(4, 24 * c), min(132, 24 * c + 24)
            nc.sync.dma_start(x2t[:, q0:q1, 4:4 + W],
                              x2d.ap()[:, q0 - 4:q1 - 4, :])

        for c in range(NG):
            load_chunks(c)

        ev = 0
        wd = 0
        for i in range(NG * nrep):
            g = i % NG
            for h in range(NHALF):
                stt = stpool.tile([128, RSH], band_dt, tag=f"st{h}")
                stv = stt[:].rearrange("p (r b u) -> p r b u", r=R, b=WBH)
                for wpl in range(WBH // PSGRP):
                    pst = pspool.tile([128, PSGRP, 512], f32, tag="ps")
                    for k in range(PSGRP):
                        wb = h * WBH + wpl * PSGRP + k
                        lhsT = x1t[:, wb, 96 * g:96 * g + 128]
                        rhs = x2t[:, 24 * g:24 * g + R, wb * WW:wb * WW + UB]
                        if mm_dtype_s == "float32r":
                            lhsT = lhsT.bitcast(mybir.dt.float32r)
                            rhs = rhs.bitcast(mybir.dt.float32r)
                        nc.tensor.matmul(pst[:, k, 0:NMM], lhsT, rhs,
                                         start=True, stop=True)
                    src = pst[:, :, 0:NMM].rearrange(
                        "p a (r u) -> p r a u", r=R).copy()
                    dst = stv[:, :, wpl * PSGRP:(wpl + 1) * PSGRP, :]
                    e = evac[ev % len(evac)]
                    ev += 1
                    if e == "v":
                        nc.vector.tensor_copy(dst, src)
                    elif e == "s":
                        nc.scalar.copy(dst, src)
                    else:
                        nc.gpsimd.tensor_copy(dst, src)
                # band window DMAs for this chunk: WINR consecutive rr per
                # DMA (pure strides only — mixed-stride APs break the tile
                # dep tracker), issue rotated across engines.
                rowlen = WBH * UB
                for q in range(NQ):
                    rr0 = q * WINR
                    src = bass.AP(stt[:].tensor,
                                  (4 * rr0) * RSH + rr0 * rowlen,
                                  [[RSH, WQP], [1, WQE_H]])
                    dst = bass.AP(outd.ap().tensor,
                                  ((g * NHALF + h) * NQ + q) * WQP * WQE_H,
                                  [[WQE_H, WQP], [1, WQE_H]])
                    e = wdma[wd % len(wdma)]
                    wd += 1
                    if e == "y":
                        nc.sync.dma_start(dst, src)
                    elif e == "s":
                        nc.scalar.dma_start(dst, src)
                    else:
                        nc.gpsimd.dma_start(dst, src)
            if i + NG < NG * nrep:
                load_chunks(g)

    nc.compile()
    return nc


def _get_nc():
    key = (IN_DTYPE, MM_DTYPE, BAND_DTYPE, EVAC, WDMA, X1_MODE)
    if key not in _CACHE:
        _CACHE[key] = _build(*key)
    return _CACHE[key]


def _prep_inputs(x1, x2):
    import ml_dtypes
    np_dt = ml_dtypes.bfloat16 if IN_DTYPE == "bfloat16" else np.float32
    in_maps = []
    for b in range(x1.shape[0]):
        x1b = x1[b].astype(np_dt)
        x1s = np.ascontiguousarray(
            x1b.reshape(128, H, NWB, WW).transpose(0, 2, 1, 3)
            .reshape(128, NWB, H * WW))
        x2n = np.ascontiguousarray(x2[b].astype(np_dt))
        in_maps.append({"x1s": x1s, "x2n": x2n})
    return in_maps


def _decode(band, out81):
    """band: per-core [OUTSZ] -> out81 [81, H, W] (scaled later)."""
    raw = np.asarray(band, np.float32).reshape(
        NG, NHALF, NQ, WQP // 4, WW, WINR, WBH, UB)
    bv = np.empty((NG, NHALF, NQ, WINR, 9, WW, WBH, UB), np.float32)
    for b in range(WINR):
        bv[:, :, :, b] = raw[:, :, :, b:b + 9, :, b]
    arr = bv.transpose(0, 2, 3, 4, 5, 1, 6, 7).reshape(NG, R, 9, WW, NWB, UB)
    for ww in range(WW):
        sub = arr[:, :, :, ww, :, ww:ww + 9]          # (g, rr, t, wb, dj)
        tmat = sub.transpose(2, 4, 0, 1, 3).reshape(9, 9, NG * R, NWB)
        for t in range(9):
            di_idx = 8 - t                             # di = 4 - t
            r2lo = di_idx
            out81[di_idx * 9:di_idx * 9 + 9, :, ww::WW] = \
                tmat[t, :, r2lo:r2lo + H, :]
    return out81


def kernel(x1, x2):
    from concourse.bass_utils import run_bass_kernel_spmd

    x1 = np.asarray(x1, np.float32)
    x2 = np.asarray(x2, np.float32)
    nc = _get_nc()
    in_maps = _prep_inputs(x1, x2)
    res = run_bass_kernel_spmd(nc, in_maps, core_ids=list(range(8)))

    inv_sqrt_c = np.float32(1.0 / math.sqrt(C))
    out = np.empty((B, NDISP - 1, H, W), np.float32)
    out81 = np.empty((NDISP, H, W), np.float32)
    for b in range(B):
        _decode(res.results[b]["band"], out81)
        out[b] = np.delete(out81, 40, axis=0) * inv_sqrt_c
    return out
